# revision 1
# baseline (speedup 1.0000x reference)
"""Encoder-layer Bass/Tile kernel for TRN2, data-parallel over 8 cores.

Layout strategy: feature-major ("transposed") activations throughout.
Core c handles batch b = c//2, sequence half h = c%2 (1024 query tokens).
Host rotates each core's x^T so that *its* tokens are always columns
0:1024 — the program is identical across cores (pure SPMD); attention is
permutation-invariant over keys so the rotated K/V order is harmless.

All matmuls run in float32r (full PE rate, ~1e-4 rel error).

Per-core pipeline:
  per feature-half (512 features = 4 head-pairs):
    P0   V = x @ Wv for those features, token-major, with a ones column
         glued per head (softmax row-sum trick).
    P1   per head-pair: project Q^T/K^T; scores^T = K_h Q_h^T with the
         two heads packed into PE row-groups; exp on ACT (scale=1/8,
         no max-subtraction — scores are O(1) by construction);
         O^T = [V_h|1]^T E accumulated over key tiles (psum row 64 =
         softmax denominator); normalize via reciprocal + K=2 broadcast
         matmul; stream O^T pair tiles to DRAM.
  P2   out-proj + residual + LayerNorm1 (partition-dim stats via
       ones-matmuls; row broadcast via K=1 matmul).
  P3   FFN1 (Relu+bias on ACT eviction), FFN2 + residual + LayerNorm2,
       write out^T.
"""

import contextlib

import numpy as np

import concourse.bass as bass  # noqa: F401
import concourse.mybir as mybir
import concourse.tile as tile
from concourse import bacc

N_CORES = 8
P = 128
D = 1024
DFF = 4096
H = 16
HD = 64
NPAIR = 8
T_MY = 1024
T_KV = 2048
DT = D // P
FT = DFF // P
KT = T_KV // P
EPS = 1e-5
B, S = 4, 2048

F32 = mybir.dt.float32
FR = mybir.dt.float32r
AF = mybir.ActivationFunctionType
ALU = mybir.AluOpType

DEFAULT_FLAGS = {
    "bq": False, "bk": False, "bv": False, "bo": False,
    "b1": False, "b2": False, "ln1": False, "ln2": False,
}


def build_encoder(flags=None, hw_reps=1, phases='all'):
    f = dict(DEFAULT_FLAGS)
    if flags:
        f.update(flags)

    nc = bacc.Bacc("TRN2", target_bir_lowering=False, debug=False,
                   num_devices=N_CORES)

    xT = nc.dram_tensor("xT", [D, T_KV], FR, kind="ExternalInput")
    Wq = nc.dram_tensor("Wq", [D, D], FR, kind="ExternalInput")
    Wk = nc.dram_tensor("Wk", [D, D], FR, kind="ExternalInput")
    Wv = nc.dram_tensor("Wv", [D, D], FR, kind="ExternalInput")
    Wo = nc.dram_tensor("Wo", [D, D], FR, kind="ExternalInput")
    W1 = nc.dram_tensor("W1", [D, DFF], FR, kind="ExternalInput")
    W2 = nc.dram_tensor("W2", [DFF, D], FR, kind="ExternalInput")
    bq = nc.dram_tensor("bq", [D], F32, kind="ExternalInput")
    bk = nc.dram_tensor("bk", [D], F32, kind="ExternalInput")
    bv = nc.dram_tensor("bv", [D], FR, kind="ExternalInput")
    bo = nc.dram_tensor("bo", [D], F32, kind="ExternalInput")
    b1 = nc.dram_tensor("b1", [DFF], F32, kind="ExternalInput")
    b2 = nc.dram_tensor("b2", [D], F32, kind="ExternalInput")
    g1 = nc.dram_tensor("g1", [D], F32, kind="ExternalInput")
    be1 = nc.dram_tensor("be1", [D], F32, kind="ExternalInput")
    g2 = nc.dram_tensor("g2", [D], F32, kind="ExternalInput")
    be2 = nc.dram_tensor("be2", [D], F32, kind="ExternalInput")
    outT = nc.dram_tensor("outT", [D, T_MY], F32, kind="ExternalOutput")
    ot_dram = nc.dram_tensor("ot_dram", [NPAIR, P, T_MY], FR)

    tensors = dict(
        xT=xT, Wq=Wq, Wk=Wk, Wv=Wv, Wo=Wo, W1=W1, W2=W2, bq=bq, bk=bk,
        bv=bv, bo=bo, b1=b1, b2=b2, g1=g1, be1=be1, g2=g2, be2=be2,
        outT=outT, ot_dram=ot_dram)

    with tile.TileContext(nc) as tc:
        if hw_reps > 1:
            with tc.For_i(0, hw_reps, 1):
                _body(nc, tc, tensors, f, phases)
        else:
            _body(nc, tc, tensors, f, phases)
    nc.compile()
    return nc


def _body(nc, tc, t, f, phases='all'):
    xT, Wq, Wk, Wv, Wo, W1, W2 = (t[k] for k in
                                  ("xT", "Wq", "Wk", "Wv", "Wo", "W1", "W2"))
    bq, bk, bv, bo, b1, b2 = (t[k] for k in ("bq", "bk", "bv", "bo", "b1",
                                             "b2"))
    g1, be1, g2, be2 = (t[k] for k in ("g1", "be1", "g2", "be2"))
    outT, ot_dram = t["outT"], t["ot_dram"]

    ctx = contextlib.ExitStack()
    with ctx:
        ctx.enter_context(nc.allow_low_precision(
            reason="float32r tiles are fp32-width; rounding is intended"))
        const = ctx.enter_context(tc.tile_pool(name="const", bufs=1))
        onesF = const.tile([P, P], F32)
        nc.vector.memset(onesF[:], 1.0)
        ones_c = const.tile([P, 1], FR)
        nc.vector.tensor_copy(ones_c[:], onesF[:, 0:1])
        ones_r = const.tile([1, P], FR)
        nc.vector.tensor_copy(ones_r[:], onesF[0:1, :])
        eps_c = const.tile([1, 1], F32)
        nc.vector.memset(eps_c[:], EPS)

        def vec_tile(pool, name, src, n):
            tl = pool.tile([P, n], F32, name=name)
            nc.sync.dma_start(tl[:], src.rearrange("(t p) -> p t", p=P))
            return tl

        bq_sb = vec_tile(const, "bq_sb", bq, DT)
        bk_sb = vec_tile(const, "bk_sb", bk, DT)
        bo_sb = vec_tile(const, "bo_sb", bo, DT)
        b1_sb = vec_tile(const, "b1_sb", b1, FT)
        b2_sb = vec_tile(const, "b2_sb", b2, DT)
        g1_sb = vec_tile(const, "g1_sb", g1, DT)
        be1_sb = vec_tile(const, "be1_sb", be1, DT)
        g2_sb = vec_tile(const, "g2_sb", g2, DT)
        be2_sb = vec_tile(const, "be2_sb", be2, DT)
        bv_sb = const.tile([1, D], FR)
        nc.sync.dma_start(bv_sb[:], bv[None, :])

        att = contextlib.ExitStack()
        with att:
            xp = att.enter_context(tc.tile_pool(name="xp", bufs=1))
            x_sb = xp.tile([P, DT, T_KV], FR)
            xTv = xT.rearrange("(kt p) t -> p kt t", p=P)
            for k in range(DT):
                nc.sync.dma_start(x_sb[:, k, :], xTv[:, k, :])
            vp = att.enter_context(tc.tile_pool(name="vp", bufs=2))
            wvp = att.enter_context(tc.tile_pool(name="wvp", bufs=2))
            wqkp = att.enter_context(tc.tile_pool(name="wqkp", bufs=2))
            qkp = att.enter_context(tc.tile_pool(name="qkp", bufs=2))
            etp = att.enter_context(tc.tile_pool(name="etp", bufs=7))
            osb = att.enter_context(tc.tile_pool(name="osb", bufs=2))
            rowp = att.enter_context(tc.tile_pool(name="rowp", bufs=1))
            ps_sc = att.enter_context(
                tc.tile_pool(name="ps_sc", bufs=3, space="PSUM"))
            ps_o = att.enter_context(
                tc.tile_pool(name="ps_o", bufs=2, space="PSUM"))
            ps_pr = ps_sc

            for grp in range(4):
                gs = slice(grp * 256, (grp + 1) * 256)
                # ---- P0: V projection for this 2-pair group ----
                wv_sb = wvp.tile([P, DT, 256], FR, tag="wv")
                nc.sync.dma_start(
                    wv_sb[:], Wv[:, gs].rearrange("(kt p) m -> p kt m", p=P))
                v_sb = vp.tile([P, KT, 2, 130], FR, tag="v")
                ones_src = onesF[:, 0:32].rearrange("p (a b o) -> p a b o",
                                                    a=KT, b=2)
                nc.vector.tensor_copy(v_sb[:, :, :, 64:65], ones_src)
                nc.vector.tensor_copy(v_sb[:, :, :, 129:130], ones_src)
                for tt in range(KT):
                    ps = ps_pr.tile([P, 256], F32, tag="sc")
                    for k in range(DT):
                        nc.tensor.matmul(
                            ps[:], x_sb[:, k, tt * P:(tt + 1) * P],
                            wv_sb[:, k, :],
                            start=(k == 0),
                            stop=(k == DT - 1 and not f["bv"]))
                    if f["bv"]:
                        nc.tensor.matmul(ps[:], ones_r[:], bv_sb[:, gs],
                                         start=False, stop=True)
                    dst = v_sb[:, tt, :, :].rearrange(
                        "p pr (hip j) -> p pr hip j", hip=2)[:, :, :, 0:64]
                    src = ps.rearrange("p (pr hip j) -> p pr hip j",
                                      pr=2, hip=2)
                    nc.vector.tensor_copy(dst, src)

                # ---- P1: the 2 head-pairs of this group ----
                for pl in range(2):
                    p = grp * 2 + pl
                    wq_p = wqkp.tile([P, DT, P], FR, tag="wq")
                    nc.sync.dma_start(
                        wq_p[:], Wq[:, p * P:(p + 1) * P]
                        .rearrange("(kt pp) m -> pp kt m", pp=P))
                    wk_p = wqkp.tile([P, DT, P], FR, tag="wk")
                    nc.sync.dma_start(
                        wk_p[:], Wk[:, p * P:(p + 1) * P]
                        .rearrange("(kt pp) m -> pp kt m", pp=P))

                    kt_sb = qkp.tile([P, 4, 512], FR, tag="kt")
                    for c in range(4):
                        ps = ps_pr.tile([P, 512], F32, tag="sc")
                        for k in range(DT):
                            nc.tensor.matmul(
                                ps[:], wk_p[:, k, :],
                                x_sb[:, k, c * 512:(c + 1) * 512],
                                start=(k == 0), stop=(k == DT - 1))
                        if f["bk"]:
                            nc.vector.tensor_scalar_add(
                                kt_sb[:, c, :], ps[:], bk_sb[:, p:p + 1])
                        else:
                            nc.vector.tensor_copy(kt_sb[:, c, :], ps[:])
                    qt_sb = qkp.tile([P, 2, 512], FR, tag="qt")
                    for c in range(2):
                        ps = ps_pr.tile([P, 512], F32, tag="sc")
                        for k in range(DT):
                            nc.tensor.matmul(
                                ps[:], wq_p[:, k, :],
                                x_sb[:, k, c * 512:(c + 1) * 512],
                                start=(k == 0), stop=(k == DT - 1))
                        if f["bq"]:
                            nc.vector.tensor_scalar_add(
                                qt_sb[:, c, :], ps[:], bq_sb[:, p:p + 1])
                        else:
                            nc.vector.tensor_copy(qt_sb[:, c, :], ps[:])

                    for qc in range(2):
                        ps_e = ps_o.tile([P, 512], F32, tag="o")
                        ps_d = ps_o.tile([P, 512], F32, tag="o")
                        for ki in range(KT):
                            sc = ps_sc.tile([P, 1024], F32, tag="sc")
                            ks = slice((ki % 4) * P, (ki % 4) * P + P)
                            nc.tensor.matmul(
                                sc[:, 0:512], kt_sb[0:HD, ki // 4, ks],
                                qt_sb[0:HD, qc, :], start=True, stop=True)
                            nc.tensor.matmul(
                                sc[:, 512:1024], kt_sb[HD:P, ki // 4, ks],
                                qt_sb[HD:P, qc, :], start=True, stop=True)
                            et = etp.tile([P, 1024], FR, tag="et")
                            nc.scalar.activation(et[:], sc[:], AF.Exp,
                                                 scale=float(1 / np.sqrt(HD)))
                            nc.tensor.matmul(
                                ps_e[0:65, :], v_sb[:, ki, pl, 0:65],
                                et[:, 0:512],
                                start=(ki == 0), stop=(ki == KT - 1))
                            nc.tensor.matmul(
                                ps_d[0:65, :], v_sb[:, ki, pl, 65:130],
                                et[:, 512:1024],
                                start=(ki == 0), stop=(ki == KT - 1))
                        qs = slice(qc * 512, (qc + 1) * 512)
                        o_e = osb.tile([65, 512], F32, tag="oe")
                        nc.vector.tensor_copy(o_e[:], ps_e[0:65, :])
                        o_d = osb.tile([65, 512], F32, tag="od")
                        nc.vector.tensor_copy(o_d[:], ps_d[0:65, :])
                        rr_e = rowp.tile([1, 512], FR, tag="rr_e")
                        nc.vector.reciprocal(rr_e[:], o_e[64:65, :])
                        rr_d = rowp.tile([1, 512], FR, tag="rr_d")
                        nc.vector.reciprocal(rr_d[:], o_d[64:65, :])
                        bc_e = ps_o.tile([P, 512], F32, tag="o")
                        nc.tensor.matmul(bc_e[0:HD, :], ones_r[:, 0:HD],
                                         rr_e[:], start=True, stop=True)
                        bc_d = ps_o.tile([P, 512], F32, tag="o")
                        nc.tensor.matmul(bc_d[0:HD, :], ones_r[:, 0:HD],
                                         rr_d[:], start=True, stop=True)
                        o_en = osb.tile([HD, 512], FR, tag="oen")
                        nc.vector.tensor_mul(o_en[:], o_e[0:HD, :],
                                             bc_e[0:HD, :])
                        o_dn = osb.tile([HD, 512], FR, tag="odn")
                        nc.vector.tensor_mul(o_dn[:], o_d[0:HD, :],
                                             bc_d[0:HD, :])
                        nc.sync.dma_start(ot_dram[p, 0:HD, qs], o_en[:])
                        nc.sync.dma_start(ot_dram[p, HD:P, qs], o_dn[:])

        if phases == "att":
            # give outT a writer so the NEFF has a valid output
            nc.sync.dma_start(outT[0:P, :], ot_dram[0].bitcast(F32))
            return
        # ======== P2: out-proj + residual + LN1 ========
        hp = ctx.enter_context(tc.tile_pool(name="hp", bufs=1))
        hT = hp.tile([P, DT, T_MY], FR)
        p2 = contextlib.ExitStack()
        with p2:
            otp = p2.enter_context(tc.tile_pool(name="otp", bufs=1))
            otr = otp.tile([P, NPAIR, T_MY], FR)
            for pr in range(NPAIR):
                nc.sync.dma_start(otr[:, pr, :], ot_dram[pr])
            wop = p2.enter_context(tc.tile_pool(name="wop", bufs=2))
            sp = p2.enter_context(tc.tile_pool(name="sp", bufs=1))
            s_sb = sp.tile([P, DT, T_MY], FR)
            xrp = p2.enter_context(tc.tile_pool(name="xrp", bufs=1))
            sqp = p2.enter_context(tc.tile_pool(name="sqp", bufs=3))
            tmpp = p2.enter_context(tc.tile_pool(name="tmpp", bufs=3))
            rwp = p2.enter_context(tc.tile_pool(name="rwp", bufs=2))
            ps_ac = p2.enter_context(
                tc.tile_pool(name="ps_ac", bufs=3, space="PSUM"))
            ps_st = p2.enter_context(
                tc.tile_pool(name="ps_st", bufs=2, space="PSUM"))
            ps_bc = p2.enter_context(
                tc.tile_pool(name="ps_bc", bufs=2, space="PSUM"))

            def ln_normalize(st_s, st_q, g_sb, be_sb, has_gb, src_of,
                             dst_of):
                mean = rwp.tile([1, 512], FR, tag="mean")
                nc.vector.tensor_scalar_mul(mean[:], st_s[:], 1.0 / D)
                msq = rwp.tile([1, 512], F32, tag="msq")
                nc.vector.tensor_scalar_mul(msq[:], st_q[:], 1.0 / D)
                m2 = rwp.tile([1, 512], F32, tag="m2")
                nc.vector.tensor_mul(m2[:], mean[:], mean[:])
                var = rwp.tile([1, 512], F32, tag="var")
                nc.vector.tensor_sub(var[:], msq[:], m2[:])
                sd = rwp.tile([1, 512], F32, tag="sd")
                nc.scalar.activation(sd[:], var[:], AF.Sqrt, bias=eps_c[:])
                rstd = rwp.tile([1, 512], FR, tag="rstd")
                nc.vector.reciprocal(rstd[:], sd[:])
                mean_b = ps_bc.tile([P, 512], F32, tag="bc")
                nc.tensor.matmul(mean_b[:], ones_r[:], mean[:],
                                 start=True, stop=True)
                rstd_b = ps_bc.tile([P, 512], F32, tag="bc")
                nc.tensor.matmul(rstd_b[:], ones_r[:], rstd[:],
                                 start=True, stop=True)
                for d in range(DT):
                    tmp = tmpp.tile([P, 512], F32, tag="tmp")
                    nc.vector.tensor_sub(tmp[:], src_of(d), mean_b[:])
                    if has_gb:
                        tmp2 = tmpp.tile([P, 512], F32, tag="tmp2")
                        nc.vector.tensor_mul(tmp2[:], tmp[:], rstd_b[:])
                        nc.vector.tensor_scalar(
                            dst_of(d), tmp2[:], g_sb[:, d:d + 1],
                            be_sb[:, d:d + 1], ALU.mult, ALU.add)
                    else:
                        nc.vector.tensor_mul(dst_of(d), tmp[:], rstd_b[:])

            for qc in range(2):
                qs = slice(qc * 512, (qc + 1) * 512)
                xr = xrp.tile([P, DT, 512], FR, tag="xr")
                nc.sync.dma_start(
                    xr[:], xT[:, qc * 512:(qc + 1) * 512]
                    .rearrange("(kt p) t -> p kt t", p=P))
                st_s = ps_st.tile([1, 512], F32, tag="st")
                st_q = ps_st.tile([1, 512], F32, tag="st")
                for d in range(DT):
                    ps = ps_ac.tile([P, 512], F32, tag="ac")
                    wo_t = wop.tile([P, DT, P], FR, tag="wo")
                    nc.sync.dma_start(
                        wo_t[:], Wo[:, d * P:(d + 1) * P]
                        .rearrange("(pr p) m -> p pr m", p=P))
                    for pr in range(NPAIR):
                        nc.tensor.matmul(ps[:], wo_t[:, pr, :],
                                         otr[:, pr, qs],
                                         start=(pr == 0),
                                         stop=(pr == NPAIR - 1))
                    if f["bo"]:
                        nc.vector.tensor_scalar_add(s_sb[:, d, qs], ps[:],
                                                    bo_sb[:, d:d + 1])
                        nc.vector.tensor_add(s_sb[:, d, qs], s_sb[:, d, qs],
                                             xr[:, d, :])
                    else:
                        nc.vector.tensor_add(s_sb[:, d, qs], ps[:],
                                             xr[:, d, :])
                    sq = sqp.tile([P, 512], FR, tag="sq")
                    nc.scalar.square(sq[:], s_sb[:, d, qs])
                    nc.tensor.matmul(st_s[:], ones_c[:], s_sb[:, d, qs],
                                     start=(d == 0), stop=(d == DT - 1))
                    nc.tensor.matmul(st_q[:], ones_c[:], sq[:],
                                     start=(d == 0), stop=(d == DT - 1))
                ln_normalize(st_s, st_q, g1_sb, be1_sb, f["ln1"],
                             lambda d: s_sb[:, d, qs],
                             lambda d: hT[:, d, qs])

        if phases == "p2":
            nc.sync.dma_start(outT[0:P, :], ot_dram[0].bitcast(F32))
            return
        # ======== P3: FFN ========
        p3 = contextlib.ExitStack()
        with p3:
            w1p = p3.enter_context(tc.tile_pool(name="w1p", bufs=3))
            ffp = p3.enter_context(tc.tile_pool(name="ffp", bufs=2))
            sqp = p3.enter_context(tc.tile_pool(name="sqp3", bufs=2))
            tmpp = p3.enter_context(tc.tile_pool(name="tmpp3", bufs=2))
            rwp = p3.enter_context(tc.tile_pool(name="rwp3", bufs=2))
            outp = p3.enter_context(tc.tile_pool(name="outp", bufs=2))
            ps_ac = p3.enter_context(
                tc.tile_pool(name="ps_ac3", bufs=3, space="PSUM"))
            ps_st = p3.enter_context(
                tc.tile_pool(name="ps_st3", bufs=2, space="PSUM"))
            ps_bc = p3.enter_context(
                tc.tile_pool(name="ps_bc3", bufs=2, space="PSUM"))

            w2p = p3.enter_context(tc.tile_pool(name="w2p", bufs=3))
            s2p = p3.enter_context(tc.tile_pool(name="s2p", bufs=1))
            NQ = 4  # d_ff quarters
            FQ = FT // NQ  # 8 fo-tiles per quarter
            s2 = s2p.tile([P, DT, T_MY], FR)
            for quarter in range(NQ):
                ff1q = ffp.tile([P, FQ, T_MY], FR, tag="ff1")
                for fo_l in range(FQ):
                    fo = quarter * FQ + fo_l
                    w1t = w1p.tile([P, DT, P], FR, tag="w1")
                    nc.sync.dma_start(
                        w1t[:], W1[:, fo * P:(fo + 1) * P]
                        .rearrange("(kt pp) m -> pp kt m", pp=P))
                    for qc in range(2):
                        qs = slice(qc * 512, (qc + 1) * 512)
                        ps = ps_ac.tile([P, 512], F32, tag="ac")
                        for k in range(DT):
                            nc.tensor.matmul(ps[:], w1t[:, k, :],
                                             hT[:, k, qs],
                                             start=(k == 0),
                                             stop=(k == DT - 1))
                        nc.scalar.activation(ff1q[:, fo_l, qs], ps[:],
                                             AF.Relu,
                                             bias=b1_sb[:, fo:fo + 1])
                for d in range(DT):
                    w2t = w2p.tile([P, FQ, P], FR, tag="w2")
                    nc.sync.dma_start(
                        w2t[:], W2[quarter * FQ * P:(quarter + 1) * FQ * P,
                                   d * P:(d + 1) * P]
                        .rearrange("(kt p) m -> p kt m", p=P))
                    for qc in range(2):
                        qs = slice(qc * 512, (qc + 1) * 512)
                        ps = ps_ac.tile([P, 512], F32, tag="ac")
                        for k in range(FQ):
                            nc.tensor.matmul(ps[:], w2t[:, k, :],
                                             ff1q[:, k, qs],
                                             start=(k == 0),
                                             stop=(k == FQ - 1))
                        if quarter == 0:
                            # seed the accumulator with residual (+ b2)
                            if f["b2"]:
                                nc.vector.tensor_scalar_add(
                                    s2[:, d, qs], ps[:], b2_sb[:, d:d + 1])
                                nc.vector.tensor_add(
                                    s2[:, d, qs], s2[:, d, qs], hT[:, d, qs])
                            else:
                                nc.vector.tensor_add(s2[:, d, qs], ps[:],
                                                     hT[:, d, qs])
                        else:
                            nc.vector.tensor_add(s2[:, d, qs],
                                                 s2[:, d, qs], ps[:])

            for qc in range(2):
                qs = slice(qc * 512, (qc + 1) * 512)
                st_s = ps_st.tile([1, 512], F32, tag="st")
                st_q = ps_st.tile([1, 512], F32, tag="st")
                for d in range(DT):
                    sq = sqp.tile([P, 512], FR, tag="sq")
                    nc.scalar.square(sq[:], s2[:, d, qs])
                    nc.tensor.matmul(st_s[:], ones_c[:], s2[:, d, qs],
                                     start=(d == 0), stop=(d == DT - 1))
                    nc.tensor.matmul(st_q[:], ones_c[:], sq[:],
                                     start=(d == 0), stop=(d == DT - 1))

                mean = rwp.tile([1, 512], FR, tag="mean")
                nc.vector.tensor_scalar_mul(mean[:], st_s[:], 1.0 / D)
                msq = rwp.tile([1, 512], F32, tag="msq")
                nc.vector.tensor_scalar_mul(msq[:], st_q[:], 1.0 / D)
                m2 = rwp.tile([1, 512], F32, tag="m2")
                nc.vector.tensor_mul(m2[:], mean[:], mean[:])
                var = rwp.tile([1, 512], F32, tag="var")
                nc.vector.tensor_sub(var[:], msq[:], m2[:])
                sd = rwp.tile([1, 512], F32, tag="sd")
                nc.scalar.activation(sd[:], var[:], AF.Sqrt, bias=eps_c[:])
                rstd = rwp.tile([1, 512], FR, tag="rstd")
                nc.vector.reciprocal(rstd[:], sd[:])
                mean_b = ps_bc.tile([P, 512], F32, tag="bc")
                nc.tensor.matmul(mean_b[:], ones_r[:], mean[:],
                                 start=True, stop=True)
                rstd_b = ps_bc.tile([P, 512], F32, tag="bc")
                nc.tensor.matmul(rstd_b[:], ones_r[:], rstd[:],
                                 start=True, stop=True)
                for d in range(DT):
                    tmp = tmpp.tile([P, 512], F32, tag="tmp")
                    nc.vector.tensor_sub(tmp[:], s2[:, d, qs], mean_b[:])
                    o_t = outp.tile([P, 512], F32, tag="out")
                    if f["ln2"]:
                        tmp2 = tmpp.tile([P, 512], F32, tag="tmp2")
                        nc.vector.tensor_mul(tmp2[:], tmp[:], rstd_b[:])
                        nc.vector.tensor_scalar(
                            o_t[:], tmp2[:], g2_sb[:, d:d + 1],
                            be2_sb[:, d:d + 1], ALU.mult, ALU.add)
                    else:
                        nc.vector.tensor_mul(o_t[:], tmp[:], rstd_b[:])
                    nc.sync.dma_start(outT[d * P:(d + 1) * P, qs], o_t[:])


# ---------------- host-side helpers ----------------

def shard_inputs(inputs):
    x = np.asarray(inputs["x"], dtype=np.float32)
    shared = {k: np.ascontiguousarray(np.asarray(inputs[k], np.float32))
              for k in ("Wq", "Wk", "Wv", "Wo", "W1", "W2", "bq", "bk", "bv",
                        "bo", "b1", "b2", "g1", "be1", "g2", "be2")}
    maps = []
    for c in range(N_CORES):
        b, h = c // 2, c % 2
        xTb = x[b].T
        roll = np.concatenate([xTb[:, h * T_MY:], xTb[:, :h * T_MY]], axis=1)
        m = {"xT": np.ascontiguousarray(roll)}
        m.update(shared)
        maps.append(m)
    return maps


def unshard_output(results):
    out = np.empty((B, S, D), np.float32)
    for c in range(N_CORES):
        b, h = c // 2, c % 2
        out[b, h * T_MY:(h + 1) * T_MY, :] = results[c]["outT"].T
    return out


def flags_from_inputs(inputs):
    def nz(k):
        return bool(np.any(np.asarray(inputs[k])))

    return {
        "bq": nz("bq"), "bk": nz("bk"), "bv": nz("bv"), "bo": nz("bo"),
        "b1": nz("b1"), "b2": nz("b2"),
        "ln1": nz("be1") or not np.allclose(np.asarray(inputs["g1"]), 1.0),
        "ln2": nz("be2") or not np.allclose(np.asarray(inputs["g2"]), 1.0),
    }


# ---------------- SPMD runner ----------------


import time

import jax
import numpy as np
from jax.sharding import Mesh, PartitionSpec
from jax.experimental.shard_map import shard_map

import concourse.bass2jax as b2j
import concourse.mybir as mybir


class SpmdRunner:
    def __init__(self, nc, n_cores: int):
        b2j.install_neuronx_cc_hook()
        self.nc = nc
        self.n_cores = n_cores

        partition_name = (
            nc.partition_id_tensor.name if nc.partition_id_tensor else None
        )
        in_names, out_names, out_avals, zero_outs = [], [], [], []
        for alloc in nc.m.functions[0].allocations:
            if not isinstance(alloc, mybir.MemoryLocationSet):
                continue
            name = alloc.memorylocations[0].name
            if alloc.kind == "ExternalInput":
                if name != partition_name:
                    in_names.append(name)
            elif alloc.kind == "ExternalOutput":
                shape = tuple(alloc.tensor_shape)
                dtype = mybir.dt.np(alloc.dtype)
                out_names.append(name)
                out_avals.append(jax.core.ShapedArray(shape, dtype))
                zero_outs.append(np.zeros(shape, dtype))
        self.in_names, self.out_names = in_names, out_names
        self.out_avals = out_avals
        n_params, n_outs = len(in_names), len(out_names)
        self.n_params = n_params

        all_in_names = list(in_names) + list(out_names)
        if partition_name is not None:
            all_in_names.append(partition_name)

        def _body(*args):
            operands = list(args)
            if partition_name is not None:
                operands.append(b2j.partition_id_tensor())
            outs = b2j._bass_exec_p.bind(
                *operands,
                out_avals=tuple(out_avals),
                in_names=tuple(all_in_names),
                out_names=tuple(out_names),
                lowering_input_output_aliases=(),
                sim_require_finite=True,
                sim_require_nnan=True,
                nc=nc,
            )
            return tuple(outs)

        devices = jax.devices()[:n_cores]
        self.mesh = Mesh(np.asarray(devices), ("core",))
        in_specs = (PartitionSpec("core"),) * (n_params + n_outs)
        out_specs = (PartitionSpec("core"),) * n_outs
        # No donation: keeps zero-out buffers reusable across repeated calls.
        self.fn = jax.jit(
            shard_map(
                _body,
                mesh=self.mesh,
                in_specs=in_specs,
                out_specs=out_specs,
                check_rep=False,
            ),
            keep_unused=True,
        )
        self.zero_outs = zero_outs
        self._dev_zeros = None

    def put_inputs(self, in_maps: list[dict[str, np.ndarray]]):
        """Concat per-core inputs on axis 0 and move to device once."""
        concat = [
            np.concatenate(
                [np.asarray(in_maps[c][n]) for c in range(self.n_cores)], axis=0
            )
            for n in self.in_names
        ]
        sharding = jax.sharding.NamedSharding(self.mesh, PartitionSpec("core"))
        dev_in = [jax.device_put(a, sharding) for a in concat]
        if self._dev_zeros is None:
            self._dev_zeros = [
                jax.device_put(
                    np.zeros((self.n_cores * z.shape[0], *z.shape[1:]), z.dtype),
                    sharding,
                )
                for z in self.zero_outs
            ]
        return dev_in

    def run(self, dev_in):
        outs = self.fn(*dev_in, *self._dev_zeros)
        jax.block_until_ready(outs)
        return outs

    def run_numpy(self, in_maps):
        dev_in = self.put_inputs(in_maps)
        outs = self.run(dev_in)
        res = []
        for c in range(self.n_cores):
            d = {}
            for i, name in enumerate(self.out_names):
                full = np.asarray(outs[i])
                per = full.reshape(self.n_cores, *self.out_avals[i].shape)
                d[name] = per[c]
            res.append(d)
        return res

    def time_runs(self, dev_in, n=10, warmup=2):
        for _ in range(warmup):
            self.run(dev_in)
        times = []
        for _ in range(n):
            t0 = time.perf_counter()
            self.run(dev_in)
            times.append(time.perf_counter() - t0)
        return times


# ---------------- public entry point ----------------

_CACHE = {}


def _get_runner(flag_key, flags):
    if flag_key not in _CACHE:
        nc = build_encoder(flags)
        _CACHE[flag_key] = SpmdRunner(nc, N_CORES)
    return _CACHE[flag_key]


def kernel(**inputs):
    """Full-input encoder layer on 8 NeuronCores; returns [B, S, D] f32."""
    flags = flags_from_inputs(inputs)
    key = tuple(sorted(flags.items()))
    in_maps = shard_inputs(inputs)
    try:
        runner = _get_runner(key, flags)
        results = runner.run_numpy(in_maps)
    except Exception:
        # Device/mesh hiccup: reset backends and retry once from scratch.
        _CACHE.clear()
        try:
            jax.clear_caches()
        except Exception:
            pass
        try:
            jax.extend.backend.clear_backends()
        except Exception:
            pass
        runner = _get_runner(key, flags)
        results = runner.run_numpy(in_maps)
    return unshard_output(results)

